# revision 5
# baseline (speedup 1.0000x reference)
"""Conv1dLoRA Trainium2 kernel.

Computes, per sample s:
  A[s] = MLP_A(a_emb[s]) in [64, 8]   (Linear-BN-GELU-Linear)
  B[s] = MLP_B(b_emb[s]) in [8, 192]
  W[s] = A[s] @ B[s]  -> per-sample conv weight [cin=64, cout*K=192]
  Y[s] = conv1d(X[s], W[s]*SCALE + base_w, pad=1) + base_b

Sharding: data-parallel over batch. 128 samples -> 16 per core x 8 cores.
MLP/base params are replicated; small host-side numpy transposes put every
weight into the exact SBUF layout the PE needs (no on-device transposes).

The kernel is HBM-DMA-bound (~358 GB/s/core), so all large streams are
bf16: X is cast f32->bf16 on the host before upload (halves the read
stream), Y is produced bf16 on device and cast back to f32 on the host
(halves the write stream). MLP weights upload as bf16 too. This cuts
per-core DMA from ~67 MB to ~35 MB.

Device program (identical SPMD program on all 8 cores):
  - MLPs batched over the 16 local samples; BN+GELU fused into one ACT op.
    pack2 (layer-2 weights, 1MB) is split into two DMAs so the hc=0
    accumulation half starts ~1.5us earlier.
  - Per-sample W via col-tiled matmuls: stg holds A[r,i]|B[r,(k,c)] for
    sample j of a pair at partitions j*32+r, so the two per-sample
    K=32 matmuls land in opposite PE quadrants and run concurrently;
    a single DVE add folds base_w in (base conv + lora conv = ONE conv).
    The staging scatter is 16 thin DMAs (8 descriptors x 512B each) on
    the scalar HWDGE ring, which is idle after the const loads.
  - Conv: per 512-col chunk, 3 taps x 2 samples as 6 quadrant-tiled
    matmuls (j picks rows/cols 0-63 vs 64-127) accumulating in PSUM.
    Zero-padded halo columns come pre-padded from the host.
  - Bias add fused into the PSUM->SBUF copy (DVE / ACT alternating),
    output written bf16. The final output block is stored in 4 slices so
    the drain tail after the last matmul is ~0.8us instead of ~3us.
  - A few dummy matmuls on already-loaded consts bridge the PE-idle gap
    between the MLP and the conv so the HAM clock gate stays at 8/8.
"""

import numpy as np
import ml_dtypes

BS, CIN, COUT, L = 128, 64, 64, 8192
K, R, GROUPS = 3, 8, 1
EMB, HID = 256, 256
BN_EPS = 1e-5
NCORES = 8
SH = BS // NCORES          # 16 samples per core
NPAIR = SH // 2            # 8 sample pairs per core
LCH = 512                  # conv chunk (one PSUM bank of fp32)
NCH = L // LCH             # 16 chunks
KCO = K * COUT             # 192 = per-sample W columns (k-major: k*64+cout)

BF16 = ml_dtypes.bfloat16

_NC = None                 # cached compiled Bass program


def _build_program():
    import concourse.tile as tile
    from concourse import bacc, mybir

    f32 = mybir.dt.float32
    bf16 = mybir.dt.bfloat16
    AF = mybir.ActivationFunctionType

    nc = bacc.Bacc(
        "TRN2",
        target_bir_lowering=False,
        debug=False,
        enable_asserts=False,
        num_devices=NCORES,
    )

    def dt_in(name, shape, dt=bf16):
        return nc.dram_tensor(name, shape, dt, kind="ExternalInput").ap()

    X = dt_in("X", [SH, CIN, L + 2])             # bf16, host pre-cast AND
    # host pre-padded with one zero column on each side (conv halo), so the
    # device never touches the halo (no memsets in the DVE queue)
    # consts packed into three tensors (fewer DMAs -> MLP starts earlier):
    # pack1 = [aT0|aT1|bT0|bT1|Aw1T0|Aw1T1|Bw1T0|Bw1T1]  (128-row halves)
    # pack2a = [Aw2T0|Bw2T0], pack2b = [Aw2T1|Bw2T1] (hc halves; two DMAs
    # so layer-2 hc=0 matmuls start before the hc=1 weights land)
    P1W = 4 * SH + 4 * HID                       # 64 + 1024
    P2W = CIN * R + R * KCO                      # 512 + 1536 = 2048
    pack1 = dt_in("pack1", [128, P1W])
    pack2a = dt_in("pack2a", [128, P2W])
    pack2b = dt_in("pack2b", [128, P2W])
    vecs = dt_in("vecs", [128, 9], f32)          # gA0 gA1 cA0 cA1 gB0 gB1 cB0 cB1 bias
    b2A = dt_in("b2A", [1, CIN * R])             # layer-2 bias rows (permuted)
    b2B = dt_in("b2B", [1, R * KCO])
    base2 = dt_in("base2", [128, KCO], f32)      # per-sample-layout base_w (x2)
    Y = nc.dram_tensor("Y", [SH, COUT, L], bf16, kind="ExternalOutput").ap()

    with tile.TileContext(nc) as tc:
        with (
            tc.tile_pool(name="const", bufs=1) as const,
            # conv-phase pools opened first so their SBUF/PSUM addresses are
            # never reused from transient pools (address reuse would add
            # write-after-read deps that stall the conv stream)
            tc.tile_pool(name="yps", bufs=3, space="PSUM") as yps,
            tc.tile_pool(name="xpool", bufs=5) as xpool,
            tc.tile_pool(name="ypool", bufs=5) as ypool,
            tc.tile_pool(name="wpool", bufs=NPAIR) as wpool,
            tc.tile_pool(name="wps", bufs=2, space="PSUM") as wps,
        ):
            # ---- constants; MLP-critical ones first, on the SYNC queue AHEAD
            # ---- of the X loads (HWDGE rings are FIFO, so consts land in
            # ---- ~4.5us and the MLP/W-gen critical path starts immediately;
            # ---- X loads have ~10us/pair of slack so delaying them is free)
            def load(name, src_ap, shape, dt=bf16, eng=None):
                t = const.tile(list(shape), dt, name=name, tag=name)
                (eng or nc.sync).dma_start(t[:], src_ap)
                return t

            pk1 = load("pack1", pack1, (128, P1W))
            vecs_sb = load("vecs", vecs, (128, 9), dt=f32)
            pk2a = load("pack2a", pack2a, (128, P2W))
            pk2b = load("pack2b", pack2b, (128, P2W))
            aT_sb = [pk1[:, e * SH:(e + 1) * SH] for e in range(2)]
            bT_sb = [pk1[:, (2 + e) * SH:(3 + e) * SH] for e in range(2)]
            o1 = 4 * SH
            Aw1T_sb = [pk1[:, o1 + e * HID:o1 + (e + 1) * HID] for e in range(2)]
            Bw1T_sb = [pk1[:, o1 + (2 + e) * HID:o1 + (3 + e) * HID] for e in range(2)]
            pk2 = [pk2a, pk2b]
            Aw2T_sb = [pk2[h][:, 0:CIN * R] for h in range(2)]
            Bw2T_sb = [pk2[h][:, CIN * R:P2W] for h in range(2)]
            gA_sb = [vecs_sb[:, h:h + 1] for h in range(2)]
            cA_sb = [vecs_sb[:, 2 + h:3 + h] for h in range(2)]
            gB_sb = [vecs_sb[:, 4 + h:5 + h] for h in range(2)]
            cB_sb = [vecs_sb[:, 6 + h:7 + h] for h in range(2)]
            bias_sb = vecs_sb[:, 8:9]
            b2A_sb = load("b2A", b2A, (1, CIN * R))
            b2B_sb = load("b2B", b2B, (1, R * KCO))
            base_sb = load("base2", base2, (128, KCO), dt=f32)
            ones_sb = const.tile([1, SH], bf16, name="ones", tag="ones")
            nc.vector.memset(ones_sb[:], 1.0)

            # MLP output, A and B interleaved per rank r so one contiguous
            # 256-col block per r feeds one staging DMA:
            # AB_row[s, r*256 + 0..63]    = A[s][i, r]      (i = col)
            # AB_row[s, r*256 + 64..255]  = B[s][r, k*64+c] (k-major)
            AB_row = const.tile([SH, 256 * R], bf16, name="AB_row", tag="AB_row")

            # W-gen staging: all pairs in one tile; sample j of pair t gets
            # stg[j*32 + r, t*256 + 0..63]   = A[s][i, r]    (i = col idx)
            # stg[j*32 + r, t*256 + 64..255] = B[s][r, k*64+c]
            # rows 8..31 / 40..63 stay zero (memset once) so a K=32 matmul
            # per sample contracts over [A-rows | zeros].
            stg_sb = const.tile([64, 256 * SH // 2 * 2], bf16, name="stg", tag="stg")
            nc.vector.memset(stg_sb[:], 0.0)

            # ---- MLPs (batched over the 16 local samples); PSUM shares the
            # ---- yps pool slots (tag "yp"), recycled before conv needs them
            gel = {}
            for side, w1T, embT, g_sb, c_sb in (
                ("A", Aw1T_sb, aT_sb, gA_sb, cA_sb),
                ("B", Bw1T_sb, bT_sb, gB_sb, cB_sb),
            ):
                for hc in range(2):
                    ps1 = yps.tile([128, SH], f32, name=f"ps1{side}{hc}", tag="yp")
                    for ec in range(2):
                        nc.tensor.matmul(
                            ps1[:],
                            w1T[ec][:, hc * 128:(hc + 1) * 128],
                            embT[ec][:],
                            start=(ec == 0),
                            stop=(ec == 1),
                        )
                    g = const.tile([128, SH], bf16, name=f"gel{side}{hc}", tag=f"gel{side}{hc}")
                    # gelu(h * g' + (b1*g' + beta)) == BN+bias+GELU fused
                    nc.scalar.activation(
                        g[:], ps1[:], AF.Gelu, bias=c_sb[hc][:], scale=g_sb[hc][:]
                    )
                    gel[(side, hc)] = g

            # layer-2: A in one 512 block (m' = r*64+i), B in 384-col blocks
            # (two ranks each); casts land strided into the interleaved
            # AB_row layout (A at r*256, B at r*256+64)
            AB4 = AB_row[:].rearrange("p (r b) -> p r b", r=R)
            for side, w2T_sb, b2_sb, width, nbw in (
                ("A", Aw2T_sb, b2A_sb, CIN * R, 512),
                ("B", Bw2T_sb, b2B_sb, R * KCO, 384),
            ):
                for nb in range(width // nbw):
                    ps2 = yps.tile([SH, nbw], f32, name=f"ps2{side}{nb}", tag="yp")
                    for hc in range(2):
                        nc.tensor.matmul(
                            ps2[:],
                            gel[(side, hc)][:],
                            w2T_sb[hc][:, nb * nbw:(nb + 1) * nbw],
                            start=(hc == 0),
                            stop=False,
                        )
                    # + layer-2 bias via rank-1 ones matmul
                    nc.tensor.matmul(
                        ps2[:],
                        ones_sb[:],
                        b2_sb[:, nb * nbw:(nb + 1) * nbw],
                        start=False,
                        stop=True,
                    )
                    if side == "A":
                        dst = AB4[:, :, 0:64]                    # [16, 8, 64]
                    else:
                        dst = AB4[:, 2 * nb:2 * nb + 2, 64:256]  # [16, 2, 192]
                    nc.vector.tensor_copy(dst, ps2[:])

            # ---- staging scatter on the scalar HWDGE ring (idle after the
            # ---- const loads; on gpsimd it would trickle behind Y stores).
            # ONE thin DMA per sample: AB_row[s] = 8 contiguous 256-col rank
            # blocks -> partitions j*32+r at col t*256 (8 descriptors x 512B)
            for t in range(NPAIR):
                for j in range(2):
                    s = 2 * t + j
                    nc.scalar.dma_start(
                        stg_sb[j * 32:j * 32 + 8, t * 256:(t + 1) * 256],
                        AB_row[s:s + 1, :].rearrange("p (r q) -> p r q", r=8),
                    )

            # ---- PE keep-warm: the HAM clock gate re-throttles to 1.2 GHz
            # after ~3.4us idle; the staging wait after the MLP is right at
            # that edge, so burn it with a few matmuls on already-loaded
            # consts (pack2a) into the to-be-recycled wps PSUM slots.
            for d in range(6):
                psd = wps.tile([128, KCO], f32, name=f"warm{d}", tag="psw")
                nc.tensor.matmul(
                    psd[:], pk2a[:, 0:128], pk2a[:, 0:KCO], start=True, stop=True
                )

            # ---- conv stream; all 8 W pairs generated up front so the conv
            # ---- never bubbles at pair boundaries
            def emit_w(t):
                # per-sample W: two K=32 col-tiled matmuls land in opposite
                # PE quadrants (rows j*32, out partitions j*64) and run
                # concurrently; psw[j*64+i, k*64+c] = W_sj[i, (k,c)]
                psw = wps.tile([128, KCO], f32, name=f"psw{t}", tag="psw")
                for j in range(2):
                    nc.tensor.matmul(
                        psw[j * 64:(j + 1) * 64, :],
                        stg_sb[j * 32:j * 32 + 32, t * 256:t * 256 + 64],
                        stg_sb[j * 32:j * 32 + 32, t * 256 + 64:(t + 1) * 256],
                        start=True,
                        stop=True,
                    )
                wpk = wpool.tile([128, KCO], bf16, name=f"wpk{t}", tag="wpk")
                nc.vector.tensor_add(wpk[:], psw[:], base_sb[:])
                return wpk

            wpks = [emit_w(t) for t in range(NPAIR)]

            OB = 4096                      # output block columns (8KB descs)
            for t in range(NPAIR):
                wpk = wpks[t]
                xp = xpool.tile([128, L + 2], bf16, name=f"xp{t}", tag="xp")
                # X load split in two halves (8KB descs) on the sync HWDGE
                # queue, so the first conv chunks start after half a load and
                # load/store transfers interleave at ~3us granularity
                xh = (L + 2) // 2
                nc.sync.dma_start(xp[:, :xh], X[2 * t:2 * t + 2, :, :xh])
                nc.sync.dma_start(xp[:, xh:], X[2 * t:2 * t + 2, :, xh:])
                last_pair = t == NPAIR - 1
                for ob in range(L // OB):
                    yo = ypool.tile([128, OB], bf16, name=f"yo{t}_{ob}", tag="yo")
                    split_store = last_pair and ob == L // OB - 1
                    for h2 in range(OB // (2 * LCH)):
                        # two chunks share one 2-bank PSUM tile so each
                        # evacuation copy covers 1024 cols (half the copies)
                        yp2 = yps.tile([128, 2 * LCH], f32, name=f"yp{t}_{ob}_{h2}", tag="yp")
                        for half in range(2):
                            c = (ob * (OB // LCH)) + h2 * 2 + half
                            for k in range(K):
                                for j in range(2):
                                    # quadrant j: rows/cols 64j..64j+63; the
                                    # two samples' matmuls run concurrently
                                    nc.tensor.matmul(
                                        yp2[j * 64:(j + 1) * 64,
                                            half * LCH:(half + 1) * LCH],
                                        wpk[j * 64:(j + 1) * 64, k * 64:(k + 1) * 64],
                                        xp[j * 64:(j + 1) * 64,
                                           c * LCH + k:c * LCH + k + LCH],
                                        start=(k == 0),
                                        stop=(k == K - 1),
                                    )
                        # bias fused into the PSUM->SBUF copy, alternating
                        # DVE / ACT so neither engine is the bottleneck
                        dst = yo[:, h2 * 2 * LCH:(h2 + 1) * 2 * LCH]
                        if h2 % 2 == 0:
                            nc.vector.tensor_scalar_add(dst, yp2[:], bias_sb[:])
                        else:
                            nc.scalar.activation(
                                dst, yp2[:], AF.Identity, bias=bias_sb[:]
                            )
                        if split_store:
                            # final block: store per-1024-col slice as soon as
                            # it is evacuated, so the post-compute drain is one
                            # 0.26MB transfer instead of 1MB
                            lo = ob * OB + h2 * 2 * LCH
                            nc.gpsimd.dma_start(
                                Y[2 * t:2 * t + 2, :, lo:lo + 2 * LCH], dst
                            )
                    # one DMA per output block (3D AP covers both samples) on
                    # the otherwise-idle gpsimd SWDGE queue, so store waits
                    # never block the ACT/DVE copy streams
                    if not split_store:
                        lo, hi = ob * OB, (ob + 1) * OB
                        nc.gpsimd.dma_start(Y[2 * t:2 * t + 2, :, lo:hi], yo[:])

    nc.compile()
    return nc


def _host_prep(inputs):
    """Shared (replicated) tensors, in device layouts. Returns dict of np arrays."""
    f = np.float32
    gA_flat = (inputs["A_bn_g"] / np.sqrt(f(1.0) + f(BN_EPS))).astype(f)
    gB_flat = (inputs["B_bn_g"] / np.sqrt(f(1.0) + f(BN_EPS))).astype(f)
    cA_flat = (inputs["A_b1"] * gA_flat + inputs["A_bn_b"]).astype(f)
    cB_flat = (inputs["B_b1"] * gB_flat + inputs["B_bn_b"]).astype(f)

    # A layer-2: columns m = i*8+r  ->  m' = r*64+i (r-major)
    permA = (np.arange(R)[:, None] + np.arange(CIN)[None, :] * R).reshape(-1)  # m'[r,i] -> i*8+r
    Aw2T = np.ascontiguousarray(inputs["A_w2"].T[:, permA]).astype(BF16)
    b2A = np.ascontiguousarray(inputs["A_b2"][permA]).astype(BF16).reshape(1, CIN * R)

    # B layer-2: columns m = r*192 + cout*3 + k  ->  m' = r*192 + k*64 + cout
    m2 = (np.arange(COUT)[None, :] * K + np.arange(K)[:, None]).reshape(-1)  # m2'[k,c] -> c*3+k
    permB = (np.arange(R)[:, None] * KCO + m2[None, :]).reshape(-1)
    Bw2T = np.ascontiguousarray(inputs["B_w2"].T[:, permB]).astype(BF16)
    b2B = np.ascontiguousarray(inputs["B_b2"][permB]).astype(BF16).reshape(1, R * KCO)

    # base_w [cout, cin, k] -> per-sample W layout [i, k*64+c], stacked for
    # both pair halves: base2[j*64+i, k*64+c] = base_w[c, i, k]
    b01 = np.ascontiguousarray(
        inputs["base_w"].transpose(1, 2, 0).reshape(CIN, KCO)
    ).astype(f)
    base2 = np.concatenate([b01, b01], axis=0)

    bias_out = np.concatenate([inputs["base_b"], inputs["base_b"]]).astype(f)

    # all per-partition vectors in one tensor -> one early DMA:
    # cols = gA0 gA1 cA0 cA1 gB0 gB1 cB0 cB1 bias_out
    vecs = np.stack([
        gA_flat[:128], gA_flat[128:], cA_flat[:128], cA_flat[128:],
        gB_flat[:128], gB_flat[128:], cB_flat[:128], cB_flat[128:],
        bias_out,
    ], axis=1).astype(f)

    # w1 halves for pack1; pack2{a,b} = [Aw2T{hc}|Bw2T{hc}]
    Aw1T = inputs["A_w1"].T.astype(BF16)
    Bw1T = inputs["B_w1"].T.astype(BF16)
    w1pack = np.concatenate(
        [Aw1T[:128], Aw1T[128:], Bw1T[:128], Bw1T[128:]], axis=1
    )
    pack2a = np.ascontiguousarray(np.concatenate([Aw2T[:128], Bw2T[:128]], axis=1))
    pack2b = np.ascontiguousarray(np.concatenate([Aw2T[128:], Bw2T[128:]], axis=1))
    return {
        "w1pack": w1pack,
        "pack2a": pack2a,
        "pack2b": pack2b,
        "vecs": vecs,
        "b2A": b2A,
        "b2B": b2B,
        "base2": base2,
    }


def _in_maps(inputs):
    shared = _host_prep(inputs)
    w1pack = shared.pop("w1pack")
    maps = []
    for c in range(NCORES):
        lo, hi = c * SH, (c + 1) * SH
        m = dict(shared)
        xp = np.zeros((SH, CIN, L + 2), dtype=BF16)
        xp[:, :, 1:L + 1] = inputs["X"][lo:hi].astype(BF16)
        m["X"] = xp
        aT = inputs["a_embedding"][lo:hi].T.astype(BF16)
        bT = inputs["b_embedding"][lo:hi].T.astype(BF16)
        m["pack1"] = np.ascontiguousarray(np.concatenate(
            [aT[:128], aT[128:], bT[:128], bT[128:], w1pack], axis=1
        ))
        maps.append(m)
    return maps


def run(inputs, trace=False):
    """Run the kernel; returns (Y_full, BassKernelResults)."""
    global _NC
    if _NC is None:
        _NC = _build_program()
    from concourse.bass_utils import run_bass_kernel_spmd

    res = run_bass_kernel_spmd(
        _NC, _in_maps(inputs), core_ids=list(range(NCORES)), trace=trace
    )
    Y = np.concatenate([r["Y"] for r in res.results], axis=0).astype(np.float32)
    return Y, res


def kernel(**inputs) -> np.ndarray:
    Y, _ = run(inputs, trace=False)
    return Y


# revision 6
# speedup vs baseline: 1.0494x; 1.0494x over previous
"""Conv1dLoRA Trainium2 kernel.

Computes, per sample s:
  A[s] = MLP_A(a_emb[s]) in [64, 8]   (Linear-BN-GELU-Linear)
  B[s] = MLP_B(b_emb[s]) in [8, 192]
  W[s] = A[s] @ B[s]  -> per-sample conv weight [cin=64, cout*K=192]
  Y[s] = conv1d(X[s], W[s]*SCALE + base_w, pad=1) + base_b

Sharding: data-parallel over batch. 128 samples -> 16 per core x 8 cores.
MLP/base params are replicated; small host-side numpy transposes put every
weight into the exact SBUF layout the PE needs (no on-device transposes).

The kernel is HBM-DMA-bound (~358 GB/s/core), so all large streams are
bf16: X is cast f32->bf16 on the host before upload (halves the read
stream), Y is produced bf16 on device and cast back to f32 on the host
(halves the write stream). MLP weights upload as bf16 too. This cuts
per-core DMA from ~67 MB to ~35 MB.

Queue discipline (a dma_start occupies the ISSUING engine's sequencer for
~0.6us while descriptors are generated, so placement matters):
  - sync (SP) ring: consts FIRST (FIFO, so they land in ~4.5us and the
    MLP starts immediately; X has ~10us/pair of slack), then X loads.
  - gpsimd (SWDGE): the 16 W-staging scatters, then Y stores. The Q7
    sequencer is otherwise idle, so the ~0.65us/trigger cost is free.
  - scalar (ACT) ring: nothing - its sequencer must stay free for the
    PSUM-evacuation activation ops.

Device program (identical SPMD program on all 8 cores):
  - MLPs batched over the 16 local samples; BN+GELU fused into one ACT op.
    pack2 (layer-2 weights, 1MB) is split into two DMAs so the hc=0
    accumulation half starts ~1.5us earlier.
  - W for a 2-sample pair via ONE K=16 PE matmul: lhsT [16,128] holds
    A[s0].T / A[s1].T block-diagonally, rhs [16,384] holds B[s0] / B[s1]
    with columns pre-permuted tap-major (k*128 + j*64 + cout), so the
    PSUM result is already the block-diagonal conv weight layout; a
    single DVE add folds base_w in (base conv + lora conv = ONE conv).
    All 8 pairs are generated up front, so the conv stream never bubbles.
  - Conv: per 512-col chunk, 3 shifted matmuls (taps) accumulate in PSUM,
    2 samples per matmul via the block-diagonal weights. Zero-padded halo
    columns come pre-padded from the host.
  - Bias add fused into the PSUM->SBUF copy (DVE / ACT alternating),
    output written bf16. The final output block is stored in 4 slices so
    the drain tail after the last matmul is ~0.8us instead of ~3us.
  - A few dummy matmuls on already-loaded consts bridge the PE-idle gap
    between the MLP and the conv so the HAM clock gate stays at 8/8.
"""

import numpy as np
import ml_dtypes

BS, CIN, COUT, L = 128, 64, 64, 8192
K, R, GROUPS = 3, 8, 1
EMB, HID = 256, 256
BN_EPS = 1e-5
NCORES = 8
SH = BS // NCORES          # 16 samples per core
NPAIR = SH // 2            # 8 sample pairs per core
LCH = 512                  # conv chunk (one PSUM bank of fp32)
NCH = L // LCH             # 16 chunks
KCO = K * COUT             # 192 = per-sample W columns (k-major: k*64+cout)

BF16 = ml_dtypes.bfloat16

_NC = None                 # cached compiled Bass program


def _build_program():
    import concourse.tile as tile
    from concourse import bacc, mybir

    f32 = mybir.dt.float32
    bf16 = mybir.dt.bfloat16
    AF = mybir.ActivationFunctionType

    nc = bacc.Bacc(
        "TRN2",
        target_bir_lowering=False,
        debug=False,
        enable_asserts=False,
        num_devices=NCORES,
    )

    def dt_in(name, shape, dt=bf16):
        return nc.dram_tensor(name, shape, dt, kind="ExternalInput").ap()

    X = dt_in("X", [SH, CIN, L + 2])             # bf16, host pre-cast AND
    # host pre-padded with one zero column on each side (conv halo), so the
    # device never touches the halo (no memsets in the DVE queue)
    # consts packed into three tensors (fewer DMAs -> MLP starts earlier):
    # pack1 = [aT0|aT1|bT0|bT1|Aw1T0|Aw1T1|Bw1T0|Bw1T1]  (128-row halves)
    # pack2a = [Aw2T0|Bw2T0], pack2b = [Aw2T1|Bw2T1] (hc halves; two DMAs
    # so layer-2 hc=0 matmuls start before the hc=1 weights land)
    P1W = 4 * SH + 4 * HID                       # 64 + 1024
    P2W = CIN * R + R * KCO                      # 512 + 1536 = 2048
    pack1 = dt_in("pack1", [128, P1W])
    pack2a = dt_in("pack2a", [128, P2W])
    pack2b = dt_in("pack2b", [128, P2W])
    vecs = dt_in("vecs", [128, 9], f32)          # gA0 gA1 cA0 cA1 gB0 gB1 cB0 cB1 bias
    b2A = dt_in("b2A", [1, CIN * R])             # layer-2 bias rows (permuted)
    b2B = dt_in("b2B", [1, R * KCO])
    base_pair = dt_in("base_pair", [128, 2 * KCO], f32)  # tap-major block-diag base_w
    Y = nc.dram_tensor("Y", [SH, COUT, L], bf16, kind="ExternalOutput").ap()

    with tile.TileContext(nc) as tc:
        with (
            tc.tile_pool(name="const", bufs=1) as const,
            # conv-phase pools opened first so their SBUF/PSUM addresses are
            # never reused from transient pools (address reuse would add
            # write-after-read deps that stall the conv stream)
            tc.tile_pool(name="yps", bufs=3, space="PSUM") as yps,
            tc.tile_pool(name="xpool", bufs=5) as xpool,
            tc.tile_pool(name="ypool", bufs=5) as ypool,
            tc.tile_pool(name="wpool", bufs=NPAIR) as wpool,
            tc.tile_pool(name="wps", bufs=2, space="PSUM") as wps,
        ):
            def load(name, src_ap, shape, dt=bf16, eng=None):
                t = const.tile(list(shape), dt, name=name, tag=name)
                (eng or nc.sync).dma_start(t[:], src_ap)
                return t

            pk1 = load("pack1", pack1, (128, P1W))
            vecs_sb = load("vecs", vecs, (128, 9), dt=f32)
            pk2a = load("pack2a", pack2a, (128, P2W))
            pk2b = load("pack2b", pack2b, (128, P2W))
            aT_sb = [pk1[:, e * SH:(e + 1) * SH] for e in range(2)]
            bT_sb = [pk1[:, (2 + e) * SH:(3 + e) * SH] for e in range(2)]
            o1 = 4 * SH
            Aw1T_sb = [pk1[:, o1 + e * HID:o1 + (e + 1) * HID] for e in range(2)]
            Bw1T_sb = [pk1[:, o1 + (2 + e) * HID:o1 + (3 + e) * HID] for e in range(2)]
            pk2 = [pk2a, pk2b]
            Aw2T_sb = [pk2[h][:, 0:CIN * R] for h in range(2)]
            Bw2T_sb = [pk2[h][:, CIN * R:P2W] for h in range(2)]
            gA_sb = [vecs_sb[:, h:h + 1] for h in range(2)]
            cA_sb = [vecs_sb[:, 2 + h:3 + h] for h in range(2)]
            gB_sb = [vecs_sb[:, 4 + h:5 + h] for h in range(2)]
            cB_sb = [vecs_sb[:, 6 + h:7 + h] for h in range(2)]
            bias_sb = vecs_sb[:, 8:9]
            b2A_sb = load("b2A", b2A, (1, CIN * R))
            b2B_sb = load("b2B", b2B, (1, R * KCO))
            base_sb = load("base_pair", base_pair, (128, 2 * KCO), dt=f32)
            ones_sb = const.tile([1, SH], bf16, name="ones", tag="ones")
            nc.vector.memset(ones_sb[:], 1.0)

            # MLP output, A and B interleaved per rank r so one contiguous
            # 256-col block per r feeds one staging DMA:
            # AB_row[s, r*256 + 0..63]    = A[s][i, r]      (i = col)
            # AB_row[s, r*256 + 64..255]  = B[s][r, k*64+c] (k-major)
            AB_row = const.tile([SH, 256 * R], bf16, name="AB_row", tag="AB_row")

            # W-gen staging: one block-diagonal tile per pair (tiny).
            # stg[j*8+r, j*64+i]             = A[s_{2t+j}][i, r]
            # stg[j*8+r, 128+k*128+j*64+c]   = B[s_{2t+j}][r, k*64+c]
            # everything else stays zero.
            stg_sb = []
            for v in range(NPAIR):
                g = const.tile([2 * R, 128 + 2 * KCO], bf16, name=f"stg{v}", tag=f"stg{v}")
                nc.vector.memset(g[:], 0.0)
                stg_sb.append(g)

            # ---- MLPs (batched over the 16 local samples); PSUM shares the
            # ---- yps/wps pool slots, recycled before the conv needs them
            gel = {}
            for side, w1T, embT, g_sb, c_sb in (
                ("A", Aw1T_sb, aT_sb, gA_sb, cA_sb),
                ("B", Bw1T_sb, bT_sb, gB_sb, cB_sb),
            ):
                for hc in range(2):
                    ps1 = yps.tile([128, SH], f32, name=f"ps1{side}{hc}", tag="yp")
                    for ec in range(2):
                        nc.tensor.matmul(
                            ps1[:],
                            w1T[ec][:, hc * 128:(hc + 1) * 128],
                            embT[ec][:],
                            start=(ec == 0),
                            stop=(ec == 1),
                        )
                    g = const.tile([128, SH], bf16, name=f"gel{side}{hc}", tag=f"gel{side}{hc}")
                    # gelu(h * g' + (b1*g' + beta)) == BN+bias+GELU fused
                    nc.scalar.activation(
                        g[:], ps1[:], AF.Gelu, bias=c_sb[hc][:], scale=g_sb[hc][:]
                    )
                    gel[(side, hc)] = g

            # layer-2: A in one 512 block (m' = r*64+i), B in 384-col blocks
            # (two ranks each); casts land strided into the interleaved
            # AB_row layout (A at r*256, B at r*256+64). PSUM alternates
            # between the yp and psw pools so block n+1's matmuls never wait
            # on block n's evacuation.
            AB4 = AB_row[:].rearrange("p (r b) -> p r b", r=R)
            nps = 0
            for side, w2T_sb, b2_sb, width, nbw in (
                ("A", Aw2T_sb, b2A_sb, CIN * R, 512),
                ("B", Bw2T_sb, b2B_sb, R * KCO, 384),
            ):
                for nb in range(width // nbw):
                    pool = (yps, "yp") if nps % 2 == 0 else (wps, "psw")
                    nps += 1
                    ps2 = pool[0].tile([SH, nbw], f32, name=f"ps2{side}{nb}", tag=pool[1])
                    for hc in range(2):
                        nc.tensor.matmul(
                            ps2[:],
                            gel[(side, hc)][:],
                            w2T_sb[hc][:, nb * nbw:(nb + 1) * nbw],
                            start=(hc == 0),
                            stop=False,
                        )
                    # + layer-2 bias via rank-1 ones matmul
                    nc.tensor.matmul(
                        ps2[:],
                        ones_sb[:],
                        b2_sb[:, nb * nbw:(nb + 1) * nbw],
                        start=False,
                        stop=True,
                    )
                    if side == "A":
                        dst = AB4[:, :, 0:64]                    # [16, 8, 64]
                    else:
                        dst = AB4[:, 2 * nb:2 * nb + 2, 64:256]  # [16, 2, 192]
                    nc.vector.tensor_copy(dst, ps2[:])

            # ---- staging scatters on the gpsimd SWDGE queue, all issued
            # ---- before any Y store; the Q7 sequencer is idle here so the
            # ---- ~0.65us/trigger emission cost delays nothing else.
            # ONE DMA per sample: AB_row[s] is per-r contiguous [A(64)|B(192)]
            # blocks, scattered to 8 partitions x 4 blocks of 64 at stride
            # 128 (A at col j*64, B tap k at 128+k*128+j*64)
            for t in range(NPAIR):
                stg = stg_sb[t]
                for j in range(2):
                    s = 2 * t + j
                    nc.gpsimd.dma_start(
                        stg[j * R:(j + 1) * R, :]
                        .rearrange("p (b c2) -> p b c2", b=4)[:, :, j * 64:(j + 1) * 64],
                        AB_row[s:s + 1, :],
                    )

            # ---- PE keep-warm: the HAM clock gate re-throttles to 1.2 GHz
            # after ~3.4us idle; the staging wait after the MLP is right at
            # that edge, so burn it with a few matmuls on already-loaded
            # consts (pack2a) into the to-be-recycled wps PSUM slots.
            for d in range(6):
                psd = wps.tile([128, 2 * KCO], f32, name=f"warm{d}", tag="psw")
                nc.tensor.matmul(
                    psd[:], pk2a[:, 0:128], pk2a[:, 0:2 * KCO], start=True, stop=True
                )

            # ---- all 8 W pairs generated up front (wps PSUM pool
            # ---- double-buffers the 8 tiny matmuls) so the conv stream
            # ---- never bubbles at pair boundaries
            def emit_w(t):
                # W pair = astg.T @ bstg: one K=16 matmul; off-diagonal zeros
                # in stg keep the cross-sample blocks zero, so psw is the
                # tap-major block-diagonal weight layout directly.
                stg = stg_sb[t]
                psw = wps.tile([128, 2 * KCO], f32, name=f"psw{t}", tag="psw")
                nc.tensor.matmul(psw[:], stg[:, 0:128], stg[:, 128:], start=True, stop=True)
                wpk = wpool.tile([128, 2 * KCO], bf16, name=f"wpk{t}", tag="wpk")
                nc.vector.tensor_add(wpk[:], psw[:], base_sb[:])
                return wpk

            wpks = [emit_w(t) for t in range(NPAIR)]

            OB = 4096                      # output block columns (8KB descs)
            for t in range(NPAIR):
                wpk = wpks[t]
                xp = xpool.tile([128, L + 2], bf16, name=f"xp{t}", tag="xp")
                # X load split in two halves (8KB descs) on the sync HWDGE
                # queue, so the first conv chunks start after half a load and
                # load/store transfers interleave at ~3us granularity
                xh = (L + 2) // 2
                nc.sync.dma_start(xp[:, :xh], X[2 * t:2 * t + 2, :, :xh])
                nc.sync.dma_start(xp[:, xh:], X[2 * t:2 * t + 2, :, xh:])
                last_pair = t == NPAIR - 1
                for ob in range(L // OB):
                    yo = ypool.tile([128, OB], bf16, name=f"yo{t}_{ob}", tag="yo")
                    split_store = last_pair and ob == L // OB - 1
                    for h2 in range(OB // (2 * LCH)):
                        # two chunks share one 2-bank PSUM tile so each
                        # evacuation copy covers 1024 cols (half the copies)
                        yp2 = yps.tile([128, 2 * LCH], f32, name=f"yp{t}_{ob}_{h2}", tag="yp")
                        for half in range(2):
                            c = (ob * (OB // LCH)) + h2 * 2 + half
                            for k in range(K):
                                nc.tensor.matmul(
                                    yp2[:, half * LCH:(half + 1) * LCH],
                                    wpk[:, k * 128:(k + 1) * 128],
                                    xp[:, c * LCH + k:c * LCH + k + LCH],
                                    start=(k == 0),
                                    stop=(k == K - 1),
                                )
                        # bias fused into the PSUM->SBUF copy, alternating
                        # DVE / ACT so neither engine is the bottleneck
                        dst = yo[:, h2 * 2 * LCH:(h2 + 1) * 2 * LCH]
                        if h2 % 2 == 0:
                            nc.vector.tensor_scalar_add(dst, yp2[:], bias_sb[:])
                        else:
                            nc.scalar.activation(
                                dst, yp2[:], AF.Identity, bias=bias_sb[:]
                            )
                        if split_store:
                            # final block: store per-1024-col slice as soon as
                            # it is evacuated, so the post-compute drain is one
                            # 0.26MB transfer instead of 1MB
                            lo = ob * OB + h2 * 2 * LCH
                            nc.gpsimd.dma_start(
                                Y[2 * t:2 * t + 2, :, lo:lo + 2 * LCH], dst
                            )
                    # one DMA per output block (3D AP covers both samples) on
                    # the otherwise-idle gpsimd SWDGE queue, so store waits
                    # never block the ACT/DVE copy streams
                    if not split_store:
                        lo, hi = ob * OB, (ob + 1) * OB
                        nc.gpsimd.dma_start(Y[2 * t:2 * t + 2, :, lo:hi], yo[:])

    nc.compile()
    return nc


def _host_prep(inputs):
    """Shared (replicated) tensors, in device layouts. Returns dict of np arrays."""
    f = np.float32
    gA_flat = (inputs["A_bn_g"] / np.sqrt(f(1.0) + f(BN_EPS))).astype(f)
    gB_flat = (inputs["B_bn_g"] / np.sqrt(f(1.0) + f(BN_EPS))).astype(f)
    cA_flat = (inputs["A_b1"] * gA_flat + inputs["A_bn_b"]).astype(f)
    cB_flat = (inputs["B_b1"] * gB_flat + inputs["B_bn_b"]).astype(f)

    # A layer-2: columns m = i*8+r  ->  m' = r*64+i (r-major)
    permA = (np.arange(R)[:, None] + np.arange(CIN)[None, :] * R).reshape(-1)  # m'[r,i] -> i*8+r
    Aw2T = np.ascontiguousarray(inputs["A_w2"].T[:, permA]).astype(BF16)
    b2A = np.ascontiguousarray(inputs["A_b2"][permA]).astype(BF16).reshape(1, CIN * R)

    # B layer-2: columns m = r*192 + cout*3 + k  ->  m' = r*192 + k*64 + cout
    m2 = (np.arange(COUT)[None, :] * K + np.arange(K)[:, None]).reshape(-1)  # m2'[k,c] -> c*3+k
    permB = (np.arange(R)[:, None] * KCO + m2[None, :]).reshape(-1)
    Bw2T = np.ascontiguousarray(inputs["B_w2"].T[:, permB]).astype(BF16)
    b2B = np.ascontiguousarray(inputs["B_b2"][permB]).astype(BF16).reshape(1, R * KCO)

    # base_w [cout, cin, k] -> tap-major block-diag pair layout:
    # base_pair[j*64 + i, k*128 + j*64 + c] = base_w[c, i, k]
    base_pair = np.zeros((128, 2 * KCO), dtype=f)
    for j in range(2):
        for k in range(K):
            base_pair[j * 64:(j + 1) * 64, k * 128 + j * 64:k * 128 + j * 64 + 64] = (
                inputs["base_w"][:, :, k].T.astype(f)
            )

    bias_out = np.concatenate([inputs["base_b"], inputs["base_b"]]).astype(f)

    # all per-partition vectors in one tensor -> one early DMA:
    # cols = gA0 gA1 cA0 cA1 gB0 gB1 cB0 cB1 bias_out
    vecs = np.stack([
        gA_flat[:128], gA_flat[128:], cA_flat[:128], cA_flat[128:],
        gB_flat[:128], gB_flat[128:], cB_flat[:128], cB_flat[128:],
        bias_out,
    ], axis=1).astype(f)

    # w1 halves for pack1; pack2{a,b} = [Aw2T{hc}|Bw2T{hc}]
    Aw1T = inputs["A_w1"].T.astype(BF16)
    Bw1T = inputs["B_w1"].T.astype(BF16)
    w1pack = np.concatenate(
        [Aw1T[:128], Aw1T[128:], Bw1T[:128], Bw1T[128:]], axis=1
    )
    pack2a = np.ascontiguousarray(np.concatenate([Aw2T[:128], Bw2T[:128]], axis=1))
    pack2b = np.ascontiguousarray(np.concatenate([Aw2T[128:], Bw2T[128:]], axis=1))
    return {
        "w1pack": w1pack,
        "pack2a": pack2a,
        "pack2b": pack2b,
        "vecs": vecs,
        "b2A": b2A,
        "b2B": b2B,
        "base_pair": base_pair,
    }


def _in_maps(inputs):
    shared = _host_prep(inputs)
    w1pack = shared.pop("w1pack")
    maps = []
    for c in range(NCORES):
        lo, hi = c * SH, (c + 1) * SH
        m = dict(shared)
        xp = np.zeros((SH, CIN, L + 2), dtype=BF16)
        xp[:, :, 1:L + 1] = inputs["X"][lo:hi].astype(BF16)
        m["X"] = xp
        aT = inputs["a_embedding"][lo:hi].T.astype(BF16)
        bT = inputs["b_embedding"][lo:hi].T.astype(BF16)
        m["pack1"] = np.ascontiguousarray(np.concatenate(
            [aT[:128], aT[128:], bT[:128], bT[128:], w1pack], axis=1
        ))
        maps.append(m)
    return maps


def run(inputs, trace=False):
    """Run the kernel; returns (Y_full, BassKernelResults)."""
    global _NC
    if _NC is None:
        _NC = _build_program()
    from concourse.bass_utils import run_bass_kernel_spmd

    res = run_bass_kernel_spmd(
        _NC, _in_maps(inputs), core_ids=list(range(NCORES)), trace=trace
    )
    Y = np.concatenate([r["Y"] for r in res.results], axis=0).astype(np.float32)
    return Y, res


def kernel(**inputs) -> np.ndarray:
    Y, _ = run(inputs, trace=False)
    return Y
